# revision 16
# baseline (speedup 1.0000x reference)
"""FFM layer kernel for 8 Trainium2 NeuronCores.

Math (reference): x[B,39] = 13 dense cols + 26 sparse index cols (ints 0..99
stored as f32).  inputs[B,2613] = [dense | one_hot(sparse)], then
  linear = inputs @ w.T + b
  field  = einsum('bn,nfk->bfk', inputs, v)        # [B,39,16]
  cross  = 0.5*sum_k((sum_f field)^2 - sum_f field^2)
  out    = sigmoid(linear + cross)

Strategy: data-parallel over batch, 2048 rows/core.  On each core the one-hot
matrix is built on-device (DVE is_equal against an offset ramp), transposed
feature-major so it can be the stationary matmul operand:
  psum[128b, 625] = sum_chunks ohT_chunk[128f,128b].T @ vperm_chunk[128f,625]
with 625 output cols = 624 field cols (k-major, f-minor) + 1 linear col.
Feature rows are [1s row (bias) | 13 dense | 26*100 one-hot | pad], packed
into 21 chunks of 128 so the PE contracts at full K=128.  fp16 operands,
fp32 PSUM accumulation.  Epilogue: strided row-reduce for s[b,k], squared
row-reduce for sum field^2, sigmoid on the scalar engine.
"""

import sys

sys.path.insert(0, "/opt/trn_rl_repo")

import numpy as np
import ml_dtypes

import concourse.bass as bass
import concourse.tile as tile
from concourse import bacc, mybir
from concourse.bass_utils import run_bass_kernel_spmd

N_CORES = 8
B_FULL = 16384
BC = B_FULL // N_CORES  # 2048 rows per core
P = 128
N_DENSE = 13
N_SPARSE = 26
SPARSE_DIM = 100
N_FIELD = 39
K_DIM = 16
NCHUNK = 21
RTOT = NCHUNK * P       # 2688 padded feature rows
NFEAT = 1 + N_DENSE + N_SPARSE * SPARSE_DIM  # 2614 incl the ones row
COLS = N_FIELD * K_DIM + 1  # 625: 624 field cols + linear col
GB = 4                  # batch tiles per group (4 psum tiles = 8 banks)

F16 = mybir.dt.float16
F32 = mybir.dt.float32
I8 = mybir.dt.int8

_prog_cache = {}


def _build_program(bc):
    """One SPMD program for a batch slice of `bc` rows (all cores identical)."""
    nbt = bc // P
    ngroups = nbt // GB
    assert nbt % GB == 0
    gw = GB * P  # one-hot column width built per group

    nc = bacc.Bacc("TRN2", target_bir_lowering=False, debug=False)
    idx_d = nc.declare_dram_parameter("idxrep", [RTOT, bc], I8, isOutput=False)
    xdn_d = nc.declare_dram_parameter("xdn", [N_DENSE, bc], F16, isOutput=False)
    vp_d = nc.declare_dram_parameter("vperm", [RTOT, COLS], F16, isOutput=False)
    ramp_d = nc.declare_dram_parameter("ramp", [P, NCHUNK], F32, isOutput=False)
    y_d = nc.declare_dram_parameter("y", [bc, 1], F32, isOutput=True)

    # idx chunk sub-batches: one DMA each on the sync/scalar HWDGE queues so
    # descriptor generation overlaps; tiny first subs so chunk 0 lands early
    # and the first matmuls can start
    ISUB = [(0, 2), (2, 11), (11, NCHUNK)]
    ISUB_ENG = ("sync", "sync", "scalar")
    VSUB = [(0, 2), (2, NCHUNK)]

    with tile.TileContext(nc) as tc:
        with (
            tc.tile_pool(name="pers", bufs=1) as pers,
            tc.tile_pool(name="idxp", bufs=2) as idxp,
            tc.tile_pool(name="psum", bufs=4, space="PSUM") as psum,
            tc.tile_pool(name="epi", bufs=3) as epi,
        ):
            ramp_t = pers.tile([P, NCHUNK], F32, tag="ramp")
            nc.sync.dma_start(ramp_t[:], ramp_d[:])

            oh_t = []
            for c in range(NCHUNK):
                oh_t.append(pers.tile([P, bc], F16, tag=f"oh{c}", name=f"oh{c}"))
            y_all = pers.tile([P, nbt], F32, tag="yall")
            vp_all = pers.tile([P, NCHUNK, COLS], F16, tag="vp")

            def load_idx(g):
                c0, c1 = g * gw, (g + 1) * gw
                subs = []
                for (lo, hi), ename in zip(ISUB, ISUB_ENG):
                    eng = getattr(nc, ename)
                    it = idxp.tile([P, hi - lo, gw], I8, tag=f"idx{lo}",
                                   name="idx", bufs=2)
                    eng.dma_start(
                        it[:],
                        idx_d[lo * P:hi * P, c0:c1].rearrange(
                            "(c p) j -> p c j", p=P),
                    )
                    subs.append((lo, it))
                return subs

            # group 0 idx loads dispatch before everything else: the first
            # one-hot build gates the first matmul
            subs0 = load_idx(0)

            # PE warmup: ~4.5us of throwaway matmuls on zeroed tiles during
            # the DMA head releases the HAM clock throttle (cold PE runs at
            # 1.2GHz for the first ~3.4us of sustained activity) so the real
            # matmuls start at 2.4GHz
            wz16 = pers.tile([P, 16], F16, tag="wz16")
            wz512 = pers.tile([P, 512], F16, tag="wz512")
            nc.vector.memset(wz16[:], 0.0)
            nc.vector.memset(wz512[:], 0.0)
            wps = psum.tile([P, COLS], F32, tag="ps", name="warmps")
            for _ in range(10):
                nc.tensor.matmul(wps[0:16, 0:512], wz16[:], wz512[:],
                                 start=True, stop=True)
            for (lo, hi), eng in zip(VSUB, (nc.scalar, nc.sync)):
                eng.dma_start(
                    vp_all[:, lo:hi, :],
                    vp_d[lo * P:hi * P, :].rearrange("(c p) j -> p c j", p=P),
                )

            for g in range(ngroups):
                c0, c1 = g * gw, (g + 1) * gw
                subs = subs0 if g == 0 else load_idx(g)
                # one is_equal per chunk builds the one-hot columns
                for lo, it in subs:
                    for ci in range(it.shape[1]):
                        c = lo + ci
                        # chunk 0 rows 0..13 hold idx=-1 so this writes 0
                        # there; the bias/dense rows are overwritten right
                        # after (engines need 32-aligned start partitions).
                        nc.vector.tensor_scalar(
                            out=oh_t[c][:, c0:c1],
                            in0=it[:, ci, :],
                            scalar1=ramp_t[:, c:c + 1],
                            scalar2=None,
                            op0=mybir.AluOpType.is_equal,
                        )
                        if c == 0:
                            # rows 0..13: constant-ones (bias) + dense; issued
                            # immediately so chunk 0's matmuls aren't gated on
                            # the rest of the construction pass
                            nc.vector.memset(oh_t[0][0:1, c0:c1], 1.0)
                            nc.scalar.dma_start(
                                oh_t[0][1:1 + N_DENSE, c0:c1],
                                xdn_d[:, c0:c1])
                for b4 in range(GB):
                    bt = g * GB + b4
                    ps = psum.tile([P, COLS], F32, tag="ps")
                    for c in range(NCHUNK):
                        lhs = oh_t[c][:, bt * P:(bt + 1) * P]
                        nc.tensor.matmul(
                            ps[:, 0:512], lhs, vp_all[:, c, 0:512],
                            start=(c == 0), stop=(c == NCHUNK - 1),
                        )
                        nc.tensor.matmul(
                            ps[:, 512:COLS], lhs, vp_all[:, c, 512:COLS],
                            start=(c == 0), stop=(c == NCHUNK - 1),
                        )
                    # epilogue: s[b,k] = sum_f field, then cross + sigmoid
                    s_t = epi.tile([P, K_DIM], F32, tag="s")
                    nc.vector.tensor_reduce(
                        out=s_t[:],
                        in_=ps[:, 0:COLS - 1].rearrange("p (k f) -> p k f", f=N_FIELD),
                        axis=mybir.AxisListType.X,
                        op=mybir.AluOpType.add,
                    )
                    sq_scr = epi.tile([P, COLS - 1], F32, tag="sqscr")
                    sqsum = epi.tile([P, 1], F32, tag="sqsum")
                    nc.scalar.activation(
                        out=sq_scr[:], in_=ps[:, 0:COLS - 1],
                        func=mybir.ActivationFunctionType.Square,
                        accum_out=sqsum[:],
                    )
                    s2_scr = epi.tile([P, K_DIM], F32, tag="s2scr")
                    s2sum = epi.tile([P, 1], F32, tag="s2sum")
                    nc.scalar.activation(
                        out=s2_scr[:], in_=s_t[:],
                        func=mybir.ActivationFunctionType.Square,
                        accum_out=s2sum[:],
                    )
                    d_t = epi.tile([P, 1], F32, tag="d")
                    nc.vector.tensor_tensor(
                        out=d_t[:], in0=s2sum[:], in1=sqsum[:],
                        op=mybir.AluOpType.subtract,
                    )
                    lin_t = epi.tile([P, 1], F32, tag="lin")
                    nc.vector.tensor_copy(lin_t[:], ps[:, COLS - 1:COLS])
                    nc.scalar.activation(
                        out=y_all[:, bt:bt + 1], in_=d_t[:],
                        func=mybir.ActivationFunctionType.Sigmoid,
                        scale=0.5, bias=lin_t[:],
                    )
            nc.sync.dma_start(
                y_d[:].rearrange("(t p) o -> p (t o)", p=P), y_all[:],
            )

    nc.compile()
    return nc


def _get_program(bc):
    if bc not in _prog_cache:
        _prog_cache[bc] = _build_program(bc)
    return _prog_cache[bc]


def _prep_shared(w_weight, w_bias, v):
    """vperm[RTOT, 625] fp16 and ramp[128, 21] f32 (same on every core)."""
    vperm = np.zeros((RTOT, COLS), np.float32)
    # cols j<624: j = k*39 + f  <->  v[n, f, k];  col 624 = linear weight
    v2 = np.ascontiguousarray(v.transpose(0, 2, 1)).reshape(2613, COLS - 1)
    vperm[1:NFEAT, :COLS - 1] = v2
    vperm[1:NFEAT, COLS - 1] = w_weight[0]
    vperm[0, COLS - 1] = float(w_bias[0])
    vperm16 = vperm.astype(np.float16)

    r = np.arange(RTOT)
    in_sparse = (r >= 1 + N_DENSE) & (r < NFEAT)
    off = np.where(in_sparse, (r - (1 + N_DENSE)) % SPARSE_DIM, 0)
    ramp = off.reshape(NCHUNK, P).T.astype(np.float32)
    ramp = np.ascontiguousarray(ramp)
    s_of_r = np.where(in_sparse, (r - (1 + N_DENSE)) // SPARSE_DIM, -1)
    return vperm16, ramp, s_of_r, in_sparse


def _prep_core(x_core, s_of_r, in_sparse):
    """Per-core idxrep[RTOT, bc] fp16 and dense xdn[13, bc] fp16."""
    bc = x_core.shape[0]
    idxrep = np.full((RTOT, bc), -1, np.int8)
    cols = (N_DENSE + s_of_r[in_sparse]).astype(np.int64)
    idxrep[in_sparse] = x_core[:, cols].T.astype(np.int8)
    xdn = np.ascontiguousarray(x_core[:, :N_DENSE].T).astype(np.float16)
    return idxrep, xdn


def run(x, w_weight, w_bias, v, trace=False, trace_kwargs=None):
    x = np.asarray(x, np.float32)
    w_weight = np.asarray(w_weight, np.float32)
    w_bias = np.asarray(w_bias, np.float32)
    v = np.asarray(v, np.float32)
    assert x.shape == (B_FULL, 39), x.shape

    vperm16, ramp, s_of_r, in_sparse = _prep_shared(w_weight, w_bias, v)
    in_maps = []
    for i in range(N_CORES):
        xc = x[i * BC:(i + 1) * BC]
        idxrep, xdn = _prep_core(xc, s_of_r, in_sparse)
        in_maps.append({
            "idxrep": idxrep,
            "xdn": xdn,
            "vperm": vperm16,
            "ramp": ramp,
        })

    nc = _get_program(BC)
    res = run_bass_kernel_spmd(
        nc, in_maps, list(range(N_CORES)),
        trace=trace, **(trace_kwargs or {}),
    )
    y = np.concatenate([res.results[i]["y"] for i in range(N_CORES)], axis=0)
    return y.astype(np.float32), res


def kernel(x, w_weight, w_bias, v):
    y, _ = run(x, w_weight, w_bias, v)
    return y
